# revision 3
# baseline (speedup 1.0000x reference)
"""GRU-D cell fused Bass kernel for Trainium2 (8 NeuronCores).

Problem: nn_GRUD_cell (B=512, IN=64, H=256, O=2, T=200) with per-step
full-batch BatchNorm.

Strategy: the BatchNorm couples the whole batch at every one of the 200
sequential timesteps, so any batch-sharding needs a cross-core stats
exchange on the critical path each step. On this stack the cheapest
cross-core primitive (ncfw collectives) costs tens of microseconds,
i.e. > the whole per-step compute — so the kernel instead runs the
fully-fused recurrence (input-side precompute + recurrent chain) as a
single replicated program on all 8 cores (zero communication); the
result is taken from core 0.

Device layout: features on partitions (H=256 -> 2 groups of 128),
batch B=512 on the free dimension. Sigmoid is computed via tanh
(sigma(x) = 0.5 + 0.5*tanh(x/2)) so the whole kernel uses the single
`exp_and_others` ACT table set (no per-step table switches). BN rstd
uses the quake rsqrt seed + 3 Newton iterations on the vector engine
(ACT Rsqrt is banned for accuracy, Sqrt lives in a different table set).
"""
import sys
sys.path.insert(0, '/opt/trn_rl_repo')
import numpy as np
import concourse.bass as bass
import concourse.tile as tile
from concourse import bacc, mybir
from concourse.alu_op_type import AluOpType as Op
from concourse.bass_utils import run_bass_kernel_spmd

F32 = mybir.dt.float32
U32 = mybir.dt.uint32
I32 = mybir.dt.int32
AF = mybir.ActivationFunctionType

B, IN, H, O, T = 512, 64, 256, 2, 200
G = 2              # feature groups (H // 128)
N_CORES = 8
BN_EPS = 1e-5
MAGIC = 0x5f3759df


def _prep_inputs(inputs):
    inp = np.asarray(inputs['input'], np.float32)   # [B, 3, IN, T]
    X = inp[:, 0]; M = inp[:, 1]; D = inp[:, 2]

    def colmajor(v):  # [H] -> [128, G]
        return np.ascontiguousarray(np.asarray(v, np.float32).reshape(G, 128).T)

    def wT(w):        # [out, in] -> lhsT tiles [128(p=in%128), in//128, out]
        w = np.asarray(w, np.float32)
        t = w.T
        kin = t.shape[0] // 128
        return np.ascontiguousarray(t.reshape(kin, 128, t.shape[1]).transpose(1, 0, 2))

    def stack2(wx, wm):
        return np.ascontiguousarray(np.concatenate(
            [np.asarray(wx, np.float32).T, np.asarray(wm, np.float32).T], axis=0))

    return {
        'x_t': np.ascontiguousarray(X.transpose(2, 1, 0)),
        'm_t': np.ascontiguousarray(M.transpose(2, 1, 0)),
        'd_t': np.ascontiguousarray(D.transpose(2, 1, 0)),
        'whz': wT(inputs['w_hz']), 'whr': wT(inputs['w_hr']),
        'whh': wT(0.5 * np.asarray(inputs['w_hh'], np.float32)),
        'wxz': stack2(inputs['w_xz'], inputs['w_mz']),
        'wxr': stack2(inputs['w_xr'], inputs['w_mr']),
        'wxh': stack2(inputs['w_xh'], inputs['w_mh']),
        'wdgh': np.ascontiguousarray(np.asarray(inputs['w_dg_h'], np.float32).T),
        'wdgx': np.ascontiguousarray(np.asarray(inputs['w_dg_x'], np.float32).T),
        'why': wT(inputs['w_hy']),
        'hbz': colmajor(0.5 * np.asarray(inputs['b_z'])),
        'hbr': colmajor(0.5 * np.asarray(inputs['b_r'])),
        'bh': colmajor(inputs['b_h']),
        'nbdgh': colmajor(-np.asarray(inputs['b_dg_h'])),
        'nbdgx': np.ascontiguousarray((-np.asarray(inputs['b_dg_x'], np.float32))[:, None]),
        'hbhy': np.ascontiguousarray((0.5 * np.asarray(inputs['b_hy'], np.float32))[:, None]),
        'gam': colmajor(inputs['bn_gamma']),
        'bet': colmajor(inputs['bn_beta']),
        'xmean': np.ascontiguousarray(np.asarray(inputs['x_mean'], np.float32)[:, None]),
    }


def build_kernel(n_cores=N_CORES, t_steps=T, repeat=1, t_io=None):
    if t_io is None:
        t_io = t_steps
    nc = bacc.Bacc("TRN2", target_bir_lowering=False, debug=False,
                   num_devices=n_cores)

    dr = {}
    def din(name, shape):
        dr[name] = nc.dram_tensor(name, shape, F32, kind="ExternalInput")

    din('x_t', [t_io, IN, B]); din('m_t', [t_io, IN, B]); din('d_t', [t_io, IN, B])
    din('whz', [128, G, H]); din('whr', [128, G, H]); din('whh', [128, G, H])
    din('wxz', [128, H]); din('wxr', [128, H]); din('wxh', [128, H])
    din('wdgh', [IN, H]); din('wdgx', [IN, IN]); din('why', [128, G, O])
    din('hbz', [128, G]); din('hbr', [128, G]); din('bh', [128, G])
    din('nbdgh', [128, G]); din('nbdgx', [IN, 1]); din('hbhy', [O, 1])
    din('gam', [128, G]); din('bet', [128, G]); din('xmean', [IN, 1])

    h_out = nc.dram_tensor("h_out", [t_io, 128, G, B], F32, kind="ExternalOutput")
    y_out = nc.dram_tensor("y_out", [t_io, O, B], F32, kind="ExternalOutput")
    xi_out = nc.dram_tensor("xi_out", [t_io, IN, B], F32, kind="ExternalOutput")

    with tile.TileContext(nc) as tc:
        if repeat > 1:
            with tc.For_i(0, repeat, 1):
                _emit(nc, tc, t_steps, dr, h_out, y_out, xi_out, t_io)
        else:
            _emit(nc, tc, t_steps, dr, h_out, y_out, xi_out, t_io)
    nc.compile()
    return nc


def _emit(nc, tc, t_steps, dr, h_out, y_out, xi_out, t_io=None):
    if t_io is None:
        t_io = t_steps
    cms = []
    def pool(name, bufs, space="SBUF"):
        p = tc.tile_pool(name=name, bufs=bufs, space=space)
        cms.append(p)          # keep the context manager alive
        return p.__enter__()

    consts = pool("consts", 1)
    state = pool("state", 1)
    xin = pool("xin", 3)
    xside = pool("xside", 3)
    gtiles = pool("gtiles", 3)
    chain = pool("chain", 2)
    stats = pool("stats", 2)
    outp = pool("outp", 3)
    ppz = pool("ppz", 1, space="PSUM")
    ppr = pool("ppr", 1, space="PSUM")
    pph = pool("pph", 1, space="PSUM")
    ppg = pool("ppg", 1, space="PSUM")

    def cload(name, shape):
        t = consts.tile(shape, F32, tag=name, name=f"c_{name}")
        nc.sync.dma_start(out=t[:], in_=dr[name][:])
        return t

    whz = cload('whz', [128, G, H]); whr = cload('whr', [128, G, H])
    whh = cload('whh', [128, G, H])
    wxz = cload('wxz', [128, H]); wxr = cload('wxr', [128, H])
    wxh = cload('wxh', [128, H])
    wdgh = cload('wdgh', [IN, H]); wdgx = cload('wdgx', [IN, IN])
    why = cload('why', [128, G, O])
    hbz = cload('hbz', [128, G]); hbr = cload('hbr', [128, G])
    bh = cload('bh', [128, G]); nbdgh = cload('nbdgh', [128, G])
    nbdgx = cload('nbdgx', [IN, 1]); hbhy = cload('hbhy', [O, 1])
    gam = cload('gam', [128, G]); bet = cload('bet', [128, G])
    xmean = cload('xmean', [IN, 1])

    magic = consts.tile([128, 1], U32, tag="magic", name="magic")
    nc.vector.memset(magic[:], MAGIC)

    hn_prev = state.tile([128, G, B], F32, tag="hn0", name="hn0")
    nc.vector.memset(hn_prev[:], 0.0)
    xl = [state.tile([IN, B], F32, tag=f"xl{i}", name=f"xl{i}") for i in range(2)]
    nc.vector.memset(xl[0][:], 0.0)

    x_d, m_d, d_d = dr['x_t'], dr['m_t'], dr['d_t']

    for t in range(t_steps):
        # ---------------- input side (independent of h) --------------------
        xt = xin.tile([IN, B], F32, tag="xt", name=f"xt{t}")
        mt = xin.tile([IN, B], F32, tag="mt", name=f"mt{t}")
        dt = xin.tile([IN, B], F32, tag="dt", name=f"dt{t}")
        nc.sync.dma_start(out=xt[:], in_=x_d[t % t_io])
        nc.sync.dma_start(out=mt[:], in_=m_d[t % t_io])
        nc.sync.dma_start(out=dt[:], in_=d_d[t % t_io])
        stk = xside.tile([128, B], F32, tag="stk", name=f"stk{t}")
        nc.sync.dma_start(out=stk[IN:128, :], in_=m_d[t % t_io])

        psg = ppg.tile([128, G, B], F32, tag="pg", name=f"psg{t}")
        for g in range(G):
            nc.tensor.matmul(psg[:, g, :], wdgh[:, 128 * g:128 * (g + 1)], dt[:],
                             start=True, stop=True)
        ghe = gtiles.tile([128, G, B], F32, tag="ghe", name=f"ghe{t}")
        for g in range(G):
            nc.scalar.activation(out=ghe[:, g, :], in_=psg[:, g, :], func=AF.Exp,
                                 bias=nbdgh[:, g:g + 1], scale=-1.0)
        ght = gtiles.tile([128, G, B], F32, tag="ght", name=f"ght{t}")
        nc.gpsimd.tensor_scalar(out=ght[:], in0=ghe[:], scalar1=1.0, scalar2=None,
                                op0=Op.min)

        psx = ppg.tile([IN, B], F32, tag="pg", name=f"psx{t}")
        nc.tensor.matmul(psx[:], wdgx[:], dt[:], start=True, stop=True)
        gxe = xside.tile([IN, B], F32, tag="gxe", name=f"gxe{t}")
        nc.scalar.activation(out=gxe[:], in_=psx[:], func=AF.Exp,
                             bias=nbdgx[:], scale=-1.0)
        gx = xside.tile([IN, B], F32, tag="gx", name=f"gx{t}")
        nc.gpsimd.tensor_scalar(out=gx[:], in0=gxe[:], scalar1=1.0, scalar2=None,
                                op0=Op.min)

        xl_prev, xl_cur = xl[t % 2], xl[(t + 1) % 2]
        nc.vector.select(out=xl_cur[:], mask=mt[:].bitcast(I32), on_true=xt[:],
                         on_false=xl_prev[:])

        t1 = xside.tile([IN, B], F32, tag="t1", name=f"t1_{t}")
        nc.gpsimd.tensor_scalar(out=t1[:], in0=xl_cur[:], scalar1=xmean[:],
                                scalar2=None, op0=Op.subtract)
        t2 = xside.tile([IN, B], F32, tag="t2", name=f"t2_{t}")
        nc.vector.tensor_tensor(out=t2[:], in0=gx[:], in1=t1[:], op=Op.mult)
        wimp = xside.tile([IN, B], F32, tag="wimp", name=f"wimp{t}")
        nc.gpsimd.tensor_scalar(out=wimp[:], in0=t2[:], scalar1=xmean[:],
                                scalar2=None, op0=Op.add)
        nc.vector.select(out=stk[0:IN, :], mask=mt[:].bitcast(I32), on_true=xt[:],
                         on_false=wimp[:])
        nc.sync.dma_start(out=xi_out[t % t_io], in_=stk[0:IN, :])

        psz = ppz.tile([128, G, B], F32, tag="pz", name=f"psz{t}")
        psr = ppr.tile([128, G, B], F32, tag="pr", name=f"psr{t}")
        psh = pph.tile([128, G, B], F32, tag="ph", name=f"psh{t}")
        for g in range(G):
            nc.tensor.matmul(psz[:, g, :], wxz[:, 128 * g:128 * (g + 1)], stk[:],
                             start=True, stop=False)
            nc.tensor.matmul(psr[:, g, :], wxr[:, 128 * g:128 * (g + 1)], stk[:],
                             start=True, stop=False)
            nc.tensor.matmul(psh[:, g, :], wxh[:, 128 * g:128 * (g + 1)], stk[:],
                             start=True, stop=False)

        # ---------------- recurrence chain ---------------------------------
        hd = chain.tile([128, G, B], F32, tag="hd", name=f"hd{t}")
        nc.vector.tensor_tensor(out=hd[:], in0=ght[:], in1=hn_prev[:], op=Op.mult)

        for g in range(G):
            for k in range(G):
                nc.tensor.matmul(psr[:, g, :], whr[:, k, 128 * g:128 * (g + 1)],
                                 hd[:, k, :], start=False, stop=(k == G - 1))
        for g in range(G):
            for k in range(G):
                nc.tensor.matmul(psz[:, g, :], whz[:, k, 128 * g:128 * (g + 1)],
                                 hd[:, k, :], start=False, stop=(k == G - 1))
            for k in range(G):
                nc.tensor.matmul(psh[:, g, :], whh[:, k, 128 * g:128 * (g + 1)],
                                 hd[:, k, :], start=False, stop=False)

        tr = chain.tile([128, G, B], F32, tag="tr", name=f"tr{t}")
        for g in range(G):
            nc.scalar.activation(out=tr[:, g, :], in_=psr[:, g, :], func=AF.Tanh,
                                 bias=hbr[:, g:g + 1], scale=0.5)
        rh = chain.tile([128, G, B], F32, tag="rh", name=f"rh{t}")
        nc.vector.tensor_tensor(out=rh[:], in0=tr[:], in1=hd[:], op=Op.mult)

        for g in range(G):
            for k in range(G):
                nc.tensor.matmul(psh[:, g, :], whh[:, k, 128 * g:128 * (g + 1)],
                                 rh[:, k, :], start=False, stop=(k == G - 1))

        tz = chain.tile([128, G, B], F32, tag="tz", name=f"tz{t}")
        for g in range(G):
            nc.scalar.activation(out=tz[:, g, :], in_=psz[:, g, :], func=AF.Tanh,
                                 bias=hbz[:, g:g + 1], scale=0.5)
        z = chain.tile([128, G, B], F32, tag="z", name=f"z{t}")
        nc.vector.tensor_scalar(out=z[:], in0=tz[:], scalar1=0.5, scalar2=0.5,
                                op0=Op.mult, op1=Op.add)

        ht = chain.tile([128, G, B], F32, tag="ht", name=f"ht{t}")
        for g in range(G):
            nc.scalar.activation(out=ht[:, g, :], in_=psh[:, g, :], func=AF.Tanh,
                                 bias=bh[:, g:g + 1], scale=1.0)

        w_ = chain.tile([128, G, B], F32, tag="w_", name=f"w{t}")
        nc.vector.tensor_tensor(out=w_[:], in0=ht[:], in1=hd[:], op=Op.subtract)
        p_ = chain.tile([128, G, B], F32, tag="p_", name=f"p{t}")
        nc.vector.tensor_tensor(out=p_[:], in0=z[:], in1=w_[:], op=Op.mult)
        hr = chain.tile([128, G, B], F32, tag="hr", name=f"hr{t}")
        nc.vector.tensor_tensor(out=hr[:], in0=hd[:], in1=p_[:], op=Op.add)

        st = stats.tile([128, G, 6], F32, tag="st", name=f"st{t}")
        for g in range(G):
            nc.vector.bn_stats(out=st[:, g, :], in_=hr[:, g, :])
        mv = [stats.tile([128, 2], F32, tag=f"mv{g}", name=f"mv{g}_{t}")
              for g in range(G)]
        for g in range(G):
            nc.vector.bn_aggr(out=mv[g][:], in_=st[:, g, :])

        a_t = stats.tile([128, G], F32, tag="a_t", name=f"a{t}")
        d_t2 = stats.tile([128, G], F32, tag="d_t2", name=f"d{t}")
        for g in range(G):
            veps = stats.tile([128, 1], F32, tag=f"veps{g}", name=f"veps{g}_{t}")
            nc.vector.tensor_scalar(out=veps[:], in0=mv[g][:, 1:2], scalar1=BN_EPS,
                                    scalar2=None, op0=Op.add)
            ish = stats.tile([128, 1], U32, tag=f"ish{g}", name=f"ish{g}_{t}")
            nc.vector.tensor_scalar(out=ish[:], in0=veps[:].bitcast(U32), scalar1=1,
                                    scalar2=None, op0=Op.logical_shift_right)
            iseed = stats.tile([128, 1], U32, tag=f"isd{g}", name=f"isd{g}_{t}")
            nc.vector.tensor_tensor(out=iseed[:], in0=magic[:], in1=ish[:],
                                    op=Op.subtract)
            ys = iseed[:].bitcast(F32)
            for it in range(3):
                y2 = stats.tile([128, 1], F32, tag=f"y2{g}", name=f"y2{g}_{t}_{it}")
                nc.vector.tensor_tensor(out=y2[:], in0=ys, in1=ys, op=Op.mult)
                vy2 = stats.tile([128, 1], F32, tag=f"vy2{g}", name=f"vy2{g}_{t}_{it}")
                nc.vector.tensor_tensor(out=vy2[:], in0=veps[:], in1=y2[:], op=Op.mult)
                f_ = stats.tile([128, 1], F32, tag=f"f_{g}", name=f"f{g}_{t}_{it}")
                nc.vector.tensor_scalar(out=f_[:], in0=vy2[:], scalar1=-0.5,
                                        scalar2=1.5, op0=Op.mult, op1=Op.add)
                yn = stats.tile([128, 1], F32, tag=f"yn{g}", name=f"yn{g}_{t}_{it}")
                nc.vector.tensor_tensor(out=yn[:], in0=ys, in1=f_[:], op=Op.mult)
                ys = yn[:]
            nc.vector.tensor_tensor(out=a_t[:, g:g + 1], in0=gam[:, g:g + 1],
                                    in1=ys, op=Op.mult)
            mua = stats.tile([128, 1], F32, tag=f"mua{g}", name=f"mua{g}_{t}")
            nc.vector.tensor_tensor(out=mua[:], in0=mv[g][:, 0:1],
                                    in1=a_t[:, g:g + 1], op=Op.mult)
            nc.vector.tensor_tensor(out=d_t2[:, g:g + 1], in0=bet[:, g:g + 1],
                                    in1=mua[:], op=Op.subtract)

        hn = outp.tile([128, G, B], F32, tag="hn", name=f"hn{t}")
        for g in range(G):
            nc.vector.tensor_scalar(out=hn[:, g, :], in0=hr[:, g, :],
                                    scalar1=a_t[:, g:g + 1], scalar2=d_t2[:, g:g + 1],
                                    op0=Op.mult, op1=Op.add)
        nc.sync.dma_start(out=h_out[t % t_io], in_=hn[:])

        psy = ppz.tile([O, B], F32, tag="pz", name=f"psy{t}")
        for k in range(G):
            nc.tensor.matmul(psy[:], why[:, k, :], hn[:, k, :],
                             start=(k == 0), stop=(k == G - 1))
        ty = outp.tile([O, B], F32, tag="ty", name=f"ty{t}")
        nc.scalar.activation(out=ty[:], in_=psy[:], func=AF.Tanh,
                             bias=hbhy[:], scale=0.5)
        ysig = outp.tile([O, B], F32, tag="ysig", name=f"ysig{t}")
        nc.vector.tensor_scalar(out=ysig[:], in0=ty[:], scalar1=0.5, scalar2=0.5,
                                op0=Op.mult, op1=Op.add)
        nc.sync.dma_start(out=y_out[t % t_io], in_=ysig[:])

        hn_prev = hn


_NC_CACHE = {}


def kernel(**inputs):
    """Full (unsharded) inputs -> full outputs, matching reference()."""
    dev_in = _prep_inputs(inputs)
    if 'nc' not in _NC_CACHE:
        _NC_CACHE['nc'] = build_kernel()
    nc = _NC_CACHE['nc']
    in_maps = [dev_in for _ in range(N_CORES)]
    res = run_bass_kernel_spmd(nc, in_maps, core_ids=list(range(N_CORES)))
    r0 = res.results[0]
    h_dev = np.asarray(r0["h_out"]).reshape(T, 128, G, B)
    y_dev = np.asarray(r0["y_out"]).reshape(T, O, B)
    xi_dev = np.asarray(r0["xi_out"]).reshape(T, IN, B)
    # device layouts back to reference layouts
    hidden = np.ascontiguousarray(
        h_dev.transpose(3, 0, 2, 1).reshape(B, T, H))   # [B,T,H], f=g*128+p
    output = np.ascontiguousarray(y_dev.transpose(2, 0, 1))    # [B,T,O]
    x_tensor = np.ascontiguousarray(xi_dev.transpose(2, 1, 0))  # [B,IN,T]
    mask = np.ascontiguousarray(np.asarray(inputs['input'], np.float32)[:, 1])
    return output, hidden, x_tensor, mask
